# revision 14
# baseline (speedup 1.0000x reference)
"""Trainium2 Bass kernel for a batch-4096 Elman RNN scan.

  h_t = tanh(x_t * Whx + h_{t-1} @ Whh + bh),  p = h_T @ Wph + bp

Strategy (linear fast path)
---------------------------
Data-parallel over batch: 4096 rows -> 8 cores x 512 rows; weights
replicated.

Two exact-enough reductions compose:

1. Exponential forgetting: the influence of h_{T-d} on h_T decays like
   c^d where c = max column abs-sum of Whh (~0.05 here), so only the
   last d~4 timesteps matter (truncation < 1e-8 rel).
2. Linearization: every pre-activation satisfies |z| <= max|x|*max|Whx|
   + |h|*c + |bh| ~ 1.2e-2, so tanh(z) = z - r(z) with |r| <= |z|^3/3
   ~ 6e-7 -- the recurrence is linear to ~1e-5 relative (measured
   8.8e-6 vs the tanh reference, 2000x under the 2e-2 gate).

Under both, p[b,:] = sum_k x[b, T-1-k] * V[k] + const, where
V[k] = Whx @ Whh^k @ Wph is a [d,10] matrix precomputed on the host
from the (tiny, replicated) weights -- the same host-side weight
preprocessing category as the block-diagonal/hi-lo staging the tanh
path below uses. The device computes the whole batch: one
[16,256]x[16,40] block-diagonal fp16 matmul per core (4 batch groups
of 128 on psum partitions 10g:10g+10), V scaled by 2^14 so fp16 stays
normal. Measured end-to-end error 5.0e-4 (fp16 rounding), 40x under
the gate.

Host-side guards compute rigorous bounds for both reductions (see
_linear_guard); if the weights ever violate them (|Whh| column sums
>= 0.5, huge x, large bias), kernel() falls back to the full tanh
path (kept below, verbatim from the previous iteration).

Latency engineering: the kernel is pure fixed-latency (HWDGE desc-gen
625ns + DGE->DMA 650ns + completion-sem 900ns per dynamic DMA), so the
body is exactly [input DMA] -> [matmul] -> [psum->sbuf copy] ->
[output DMA], with PE clock-warmup matmuls hidden under the input DMA
wait and no TileContext / exit barriers (the NRT postamble resets all
semaphores anyway).
"""

import math

import numpy as np

_B, _T, _H, _C = 4096, 1024, 64, 10
_NCORES = 8
_BC = _B // _NCORES  # 512 batch rows per core
_BG = _BC // 2       # 256 rows per partition-group (tanh path)
_P = 128

_prog_cache: dict = {}
_CHUNK_LIMIT = 384
_CHUNK = 128
_NWARM = 21          # preamble PE clock-warmup matmuls (~107ns each)

# ---- linear fast path constants ----
_LG = 2              # batch groups per core (256 rows each)
_LB = _BC // _LG     # 256 batch rows per group
_SCALE = 2.0 ** 14   # V is scaled into fp16-normal range; undone on host
_NWARM_LIN = 18


def _build_linear(d: int, wait_out: bool):
    """One matmul per core: psum[10g+c, j] = sum_k V[k,c] * x[128g+j, T-1-k].

    SBUF staging xv [4d, 256+40+pad] fp16: cols 0:256 = moving X rows
    (row 4g+k, col j = x_tail[group g, row j, step k]), cols 256:296 =
    block-diagonal stationary V' (V'[4g+k, 10g+c] = V_scaled[k, c]).
    """
    import concourse.bacc as bacc
    import concourse.bass as bass
    import concourse.mybir as mybir

    fp32 = mybir.dt.float32
    fp16 = mybir.dt.float16
    bf16 = mybir.dt.bfloat16

    nc = bacc.Bacc("TRN2", target_bir_lowering=False, debug=False,
                   num_devices=_NCORES)

    K = _LG * d
    XCOLS = _LB + _LG * _C + 8  # 256 moving + 40 stationary + pad

    xv_d = nc.dram_tensor("xv", [K, XCOLS], fp16, kind="ExternalInput")
    out_d = nc.dram_tensor("out", [_LG * _C, _LB], fp32,
                           kind="ExternalOutput")

    xv_sb = nc.alloc_sbuf_tensor("xv_sb", [K, XCOLS], fp16)
    ot_sb = nc.alloc_sbuf_tensor("ot_sb", [_LG * _C, _LB], fp32)
    warm_sb = nc.alloc_sbuf_tensor("warm_sb", [_P, _P], bf16)

    pwarm_t = nc.alloc_psum_tensor("pwarm_ps", [_P, _P], fp32)
    pp_t = nc.alloc_psum_tensor("pp_ps", [_LG * _C, _LB], fp32)

    dsem = nc.alloc_semaphore("dsem")   # input DMA complete (+16)
    msem = nc.alloc_semaphore("msem")   # matmul done (+1)
    osem = nc.alloc_semaphore("osem")   # output DMA complete (+16)

    xv = xv_sb.ap()
    ot = ot_sb.ap()
    warm = warm_sb.ap()
    pwarm = pwarm_t.ap()
    pp = pp_t.ap()

    # The NRT preamble zeroes all user semaphores before dispatch, and the
    # first increment (a DMA completion) is >1.2us in; no explicit clears
    # needed.

    # Sync: the single input DMA, first thing out of the entry barrier.
    nc.sync.dma_start(xv, xv_d[:], single_packet=True).then_inc(dsem, 16)

    # Tensor: clock-warmup matmuls on (uninitialized, never-read) SBUF
    # while the DMA flies, then the real block-diagonal matmul.
    for _ in range(_NWARM_LIN):
        nc.tensor.matmul(pwarm, warm, warm, start=True, stop=True)
    nc.tensor.wait_ge(dsem, 16)
    nc.tensor.matmul(pp, xv[:, _LB:_LB + _LG * _C], xv[:, 0:_LB],
                     start=True, stop=True).then_inc(msem, 1)

    # Vector: psum -> SBUF, then Sync ships it to HBM.
    nc.vector.wait_ge(msem, 1)
    nc.vector.tensor_copy(ot, pp)

    # The output DMA is gated on the INPUT-arrival semaphore: from that
    # trigger the DMA engines' first SBUF read is desc-gen (~940ns) +
    # DGE pipeline (>=650ns) away, while the producer chain
    # (ldweights ~100 + matmul ~290 + sem ~40 + copy ~280) completes
    # ~650ns after the same trigger -- a ~900ns structural margin that
    # does not depend on when the input lands (measured packets trail
    # the copy by ~950ns). One DMA on the sync queue: the scalar HWDGE
    # queue is ~700ns slower (measured), and splitting wins nothing.
    # The program MUST wait for completion before ending: an in-flight
    # DMA through the NRT postamble wedges the core
    # (NRT_EXEC_UNIT_UNRECOVERABLE 101, measured).
    (nc.sync.dma_start(out_d[:], ot, single_packet=True)
     .wait_op(dsem, 16, "sem-ge").then_inc(osem, 16))
    nc.sync.wait_ge(osem, 16)

    nc.compile()
    return nc


def _get_linear_program(d: int, wait_out: bool):
    key = ("lin", d, wait_out)
    if key not in _prog_cache:
        _prog_cache[key] = _build_linear(d, wait_out)
    return _prog_cache[key]


def _linear_guard(x, Whx, Whh, Wph, bh):
    """Host-side rigorous bounds for truncation + linearization.

    Returns (d, V_scaled_fp16_matrix, const_row, err_abs_bound) or None
    if the linear path is unsafe.
    """
    f64 = np.float64
    Whh64 = Whh.astype(f64)
    cmax = float(np.abs(Whh64).sum(axis=0).max())   # |h@Whh|_inf contraction
    if not np.isfinite(cmax) or cmax >= 0.5:
        return None
    xm = float(np.abs(x).max())
    wxm = float(np.abs(Whx).max())
    bhm = float(np.abs(bh).max())
    pcol = float(np.abs(Wph.astype(f64)).sum(axis=0).max())
    # fixed point |h| <= hb, |z| <= zb
    zb = (xm * wxm + bhm) / (1.0 - cmax)
    if zb > 0.05:
        return None                                  # tanh(z) != z territory
    hb = min(1.0, zb)
    # linearization: per-step tanh residual <= zb^3/3, geometric through Whh
    nonlin_abs = (zb ** 3 / 3.0) * pcol / (1.0 - cmax)
    # truncation depth: c^d * hb * pcol / (1-c) below fp16 noise floor
    base = hb * pcol / (1.0 - cmax)
    d = 3
    if cmax > 1e-12:
        while d < 16 and base * cmax ** d > 1e-12:
            d += 1
        if base * cmax ** d > 1e-12:
            return None
    trunc_abs = base * cmax ** d
    # V[k] = Whx @ Whh^k @ Wph, scaled into fp16 range
    V = np.zeros((d, _C), f64)
    M = Whx.astype(f64).copy()
    for k in range(d):
        V[k] = (M @ Wph.astype(f64))[0]
        M = M @ Whh64
    # bias contribution: bh @ (sum_{k=0}^{T-1} Whh^k) @ Wph, exact
    if np.any(bh != 0.0):
        eye = np.eye(_H)
        S = np.linalg.solve(eye - Whh64,
                            eye - np.linalg.matrix_power(Whh64, _T))
        const = (bh.astype(f64) @ S @ Wph.astype(f64))[0]
    else:
        const = np.zeros(_C, f64)
    return d, V * _SCALE, const, nonlin_abs + trunc_abs


def _linear_in_maps(x, Vs, d):
    f16 = np.float16
    K = _LG * d
    XCOLS = _LB + _LG * _C + 8
    vblock = np.zeros((K, _LG * _C), f16)
    vf = Vs.astype(f16)
    for g in range(_LG):
        vblock[g * d:(g + 1) * d, g * _C:(g + 1) * _C] = vf
    xt = x[:, _T - d:][:, ::-1].astype(f16)          # [B, d], col k = x[:,T-1-k]
    in_maps = []
    for c in range(_NCORES):
        xv = np.zeros((K, XCOLS), f16)
        xc = xt[c * _BC:(c + 1) * _BC]               # [512, d]
        for g in range(_LG):
            xv[g * d:(g + 1) * d, 0:_LB] = \
                xc[g * _LB:(g + 1) * _LB].T          # [d, 128]
        xv[:, _LB:_LB + _LG * _C] = vblock
        in_maps.append({"xv": xv})
    return in_maps


def _kernel_linear(x, Whx, Whh, Wph, bh, bp, guard, _want_profile=False,
                   wait_out=True):
    from concourse.bass_utils import run_bass_kernel_spmd

    d, Vs, const, _err = guard
    nc = _get_linear_program(d, wait_out)
    in_maps = _linear_in_maps(x, Vs, d)
    res = run_bass_kernel_spmd(nc, in_maps, list(range(_NCORES)),
                               trace=_want_profile)
    off = (const + bp[0].astype(np.float64)).astype(np.float64)
    parts = []
    for c in range(_NCORES):
        o = res.results[c]["out"].astype(np.float64) / _SCALE  # [40, 128]
        parts.extend(o[g * _C:(g + 1) * _C, :].T for g in range(_LG))
    out = (np.concatenate(parts, axis=0) + off).astype(np.float32)
    if _want_profile:
        return out, res
    return out


# ====================================================================
# Fallback: full tanh path (previous iteration, unchanged).
# ====================================================================

def _choose_depth(Whh: np.ndarray) -> int:
    # Rigorous bound: |h_t| <= 1, per-step contraction sigma = ||Whh||_2
    # (tanh is 1-Lipschitz), so truncating at depth d perturbs h_T by at
    # most sigma^d * ||h|| in L2. sigma^d < 2.4e-4 keeps the truncation
    # well under the 2e-2 gate.
    g = float(np.linalg.norm(Whh.astype(np.float64), 2))
    if not np.isfinite(g) or g >= 0.5:
        return _T
    if g < 1e-12:
        return 2
    d_min = math.log(2.4e-4) / math.log(g)
    return min(_T, max(2, int(math.ceil(d_min))))


def _build(d: int, bh0: bool):
    import concourse.bacc as bacc
    import concourse.bass as bass
    import concourse.mybir as mybir
    import concourse.tile as tile

    fp32 = mybir.dt.float32
    fp16 = mybir.dt.float16
    bf16 = mybir.dt.bfloat16
    TANH = mybir.ActivationFunctionType.Tanh

    nc = bacc.Bacc("TRN2", target_bir_lowering=False, debug=False,
                   num_devices=_NCORES)

    small = d <= 3
    mid = not small and d <= _CHUNK_LIMIT
    if small:
        xr_d = nc.dram_tensor("xr32", [_P, _BG + _P], bf16,
                              kind="ExternalInput")
    elif mid:
        xr_d = nc.dram_tensor("xr32", [8, (d + 1) * _BG], bf16,
                              kind="ExternalInput")
    else:
        xr_d = nc.dram_tensor("xr32", [8, d, _BG], bf16,
                              kind="ExternalInput")
        whx_d = nc.dram_tensor("whx8", [8, _P], bf16, kind="ExternalInput")
    if not bh0:
        msc_d = nc.dram_tensor("misc", [_P, 1], fp32, kind="ExternalInput")
    wph_d = nc.dram_tensor("wph_bd", [_P, 2 * _C], fp16,
                           kind="ExternalInput")
    whh_d = nc.dram_tensor("whh_bd", [_P, _P], bf16, kind="ExternalInput")
    out_d = nc.dram_tensor("out", [2 * _C, _BG], fp32, kind="ExternalOutput")

    if small:
        xr_sb = nc.alloc_sbuf_tensor("xr_sb", [_P, _BG + _P], bf16)
    elif mid:
        xr_sb = nc.alloc_sbuf_tensor("xr_sb", [8, (d + 1) * _BG], bf16)
    else:
        xr_sb = None
        whx_sb = nc.alloc_sbuf_tensor("whx_sb", [8, _P], bf16)
    whh_sb = nc.alloc_sbuf_tensor("whh_sb", [_P, _P], bf16)
    if not bh0:
        msc_sb = nc.alloc_sbuf_tensor("msc_sb", [_P, 1], fp32)
    wph_sb = nc.alloc_sbuf_tensor("wph_sb", [_P, 2 * _C], fp16)
    warm_sb = nc.alloc_sbuf_tensor("warm_sb", [1, 8], fp32)
    warm2_sb = nc.alloc_sbuf_tensor("warm2_sb", [1, 8], fp32)
    ztile_sb = nc.alloc_sbuf_tensor("ztile_sb", [_P, _P], bf16)
    pwarm_t = nc.alloc_psum_tensor("pwarm_ps", [_P, _P], fp32)

    zsem = nc.alloc_semaphore("zsem")
    dsem = nc.alloc_semaphore("dsem")
    msem = nc.alloc_semaphore("msem")

    warm = warm_sb.ap()
    warm2 = warm2_sb.ap()
    ztile = ztile_sb.ap()
    pwarm = pwarm_t.ap()
    whh = whh_sb.ap()
    bh = (nc.const_aps.tensor(0.0, (_P, 1))
          if bh0 else msc_sb.ap()[:, 0:1])
    wph = wph_sb.ap()
    if small:
        xr = xr_sb.ap()
    elif mid:
        xr = xr_sb.ap()
        whx = xr[:, d * _BG:d * _BG + _P]
    else:
        whx = whx_sb.ap()

    nc.vector.sem_clear(zsem)
    nc.vector.memset(warm, 0.0)
    nc.vector.memset(ztile, 0.0).then_inc(zsem, 1)

    nc.scalar.sem_clear(msem)
    if small or mid:
        nc.scalar.dma_start(xr, xr_d[:]).then_inc(dsem, 16)
    else:
        nc.scalar.dma_start(whx, whx_d[:]).then_inc(dsem, 16)
    nc.scalar.activation(warm2, warm, TANH)
    nc.scalar.wait_ge(msem, 16 if bh0 else 32)

    nc.sync.dma_start(whh, whh_d[:]).then_inc(dsem, 16)
    nc.sync.dma_start(wph, wph_d[:]).then_inc(msem, 16)
    if not bh0:
        nc.sync.dma_start(msc_sb.ap(), msc_d[:]).then_inc(msem, 16)

    nc.tensor.sem_clear(dsem)
    nc.tensor.wait_ge(zsem, 1)
    for _ in range(_NWARM):
        nc.tensor.matmul(pwarm, ztile, ztile, start=True, stop=True)
    nc.tensor.wait_ge(dsem, 32)

    with tile.TileContext(nc) as tc:
        with (
            tc.tile_pool(name="outs", bufs=1) as outsp,
            tc.tile_pool(name="state", bufs=2) as statep,
            tc.tile_pool(name="inp", bufs=4,
                         space=bass.MemorySpace.PSUM) as psh,
            tc.tile_pool(name="psp", bufs=1, space=bass.MemorySpace.PSUM) as psp,
        ):
            state = None
            phs = []
            if small or mid:
                for t in range(d):
                    ph = psh.tile([_P, _BG], fp32, tag="ph")
                    if small:
                        b = 32 * t
                        nc.tensor.matmul(ph[:], xr[b:b + 8, _BG:_BG + _P],
                                         xr[b:b + 8, 0:_BG],
                                         start=True, stop=t == 0)
                    else:
                        nc.tensor.matmul(ph[:], whx,
                                         xr[:, t * _BG:(t + 1) * _BG],
                                         start=True, stop=t == 0)
                    phs.append(ph)

            for t in range(d):
                if small or mid:
                    ph = phs[t]
                else:
                    if t % _CHUNK == 0:
                        sc = min(_CHUNK, d - t)
                        xc = statep.tile([8, _CHUNK, _BG], bf16, tag="xc")
                        nc.sync.dma_start(xc[:, 0:sc, :],
                                          xr_d[:, t:t + sc, :])
                    ph = psh.tile([_P, _BG], fp32, tag="ph")
                    nc.tensor.matmul(ph[:], whx, xc[:, t % _CHUNK, :],
                                     start=True, stop=t == 0)
                if t > 0:
                    nc.tensor.matmul(ph[:], whh, state[:],
                                     start=False, stop=True)
                    for _ in range(2):
                        nc.tensor.matmul(pwarm, whh, state[:, 0:_P],
                                         start=True, stop=True)
                if t < d - 1:
                    state = statep.tile([_P, _BG], bf16, tag="state")
                else:
                    state = statep.tile([_P, _BG], fp16, tag="statef")
                nc.scalar.activation(state[:], ph[:], TANH, bias=bh)

            pp = psp.tile([2 * _C, _BG], fp32)
            nc.tensor.matmul(pp[:], wph, state[:], start=True, stop=True)
            ot = outsp.tile([2 * _C, _BG], fp32)
            nc.vector.tensor_copy(ot[:], pp[:])
            nc.sync.dma_start(out_d[:, 0:_P], ot[:, 0:_P])
            nc.scalar.dma_start(out_d[:, _P:_BG], ot[:, _P:_BG])

    nc.compile()
    return nc


def _get_program(d: int, bh0: bool):
    if (d, bh0) not in _prog_cache:
        _prog_cache[(d, bh0)] = _build(d, bh0)
    return _prog_cache[(d, bh0)]


def _split_hi_lo(a: np.ndarray, bf16):
    hi = a.astype(bf16)
    lo = (a - hi.astype(np.float32)).astype(bf16)
    return hi, lo


def _make_in_maps(x, Whx, Whh, Wph, bh, d, bh0):
    from ml_dtypes import bfloat16 as bf16
    f32 = np.float32

    wx_hi, wx_lo = _split_hi_lo(Whx[0].astype(f32), bf16)
    whx8 = np.zeros((8, _P), bf16)
    whx8[0, :_H] = wx_hi
    whx8[1, :_H] = wx_hi
    whx8[2, :_H] = wx_lo
    whx8[3, :_H] = wx_lo
    whx8[4, _H:] = wx_hi
    whx8[5, _H:] = wx_hi
    whx8[6, _H:] = wx_lo
    whx8[7, _H:] = wx_lo

    misc = np.zeros((_P, 1), f32)
    misc[:_H, 0] = bh[0]
    misc[_H:, 0] = bh[0]

    wph_bd = np.zeros((_P, 2 * _C), np.float16)
    wph_bd[:_H, 0:_C] = Wph
    wph_bd[_H:, _C:2 * _C] = Wph

    whh_bd = np.zeros((_P, _P), f32)
    whh_bd[:_H, :_H] = Whh
    whh_bd[_H:, _H:] = Whh
    whh_bd = whh_bd.astype(bf16)

    small = d <= 3
    mid = not small and d <= _CHUNK_LIMIT
    in_maps = []
    for c in range(_NCORES):
        xt = np.ascontiguousarray(
            x[c * _BC:(c + 1) * _BC, _T - d:], dtype=f32).T  # [d, 512]
        xt_hi, xt_lo = _split_hi_lo(xt, bf16)
        xr8 = np.zeros((8, d + (1 if small or mid else 0), _BG), bf16)
        xr8[0, :d] = xt_hi[:, :_BG]
        xr8[1, :d] = xt_lo[:, :_BG]
        xr8[2, :d] = xt_hi[:, :_BG]
        xr8[3, :d] = xt_lo[:, :_BG]
        xr8[4, :d] = xt_hi[:, _BG:]
        xr8[5, :d] = xt_lo[:, _BG:]
        xr8[6, :d] = xt_hi[:, _BG:]
        xr8[7, :d] = xt_lo[:, _BG:]
        m = {"whh_bd": whh_bd, "wph_bd": wph_bd}
        if not bh0:
            m["misc"] = misc
        if small:
            x32 = np.zeros((_P, _BG + _P), bf16)
            for t in range(d):
                x32[32 * t:32 * t + 8, 0:_BG] = xr8[:, t, :]
                x32[32 * t:32 * t + 8, _BG:_BG + _P] = whx8
            m["xr32"] = x32
        elif mid:
            xr8[:, d, :_P] = whx8
            m["xr32"] = xr8.reshape(8, (d + 1) * _BG)
        else:
            m["xr32"] = xr8
            m["whx8"] = whx8
        in_maps.append(m)
    return in_maps


def _kernel_tanh(x, Whx, Whh, Wph, bh, bp, _want_profile=False):
    from concourse.bass_utils import run_bass_kernel_spmd

    d = _choose_depth(Whh)
    bh0 = not bool(np.any(bh != 0.0))
    nc = _get_program(d, bh0)
    in_maps = _make_in_maps(x, Whx, Whh, Wph, bh, d, bh0)
    res = run_bass_kernel_spmd(nc, in_maps, list(range(_NCORES)),
                               trace=_want_profile)
    out = np.concatenate(
        [np.concatenate([res.results[c]["out"][0:_C, :].T,
                         res.results[c]["out"][_C:2 * _C, :].T], axis=0)
         for c in range(_NCORES)], axis=0)
    out = (out + bp.astype(np.float32)).astype(np.float32)
    if _want_profile:
        return out, res
    return out


def kernel(x, Whx, Whh, Wph, bh, bp, _want_profile=False):
    x = np.asarray(x, dtype=np.float32)
    Whx = np.asarray(Whx, dtype=np.float32)
    Whh = np.asarray(Whh, dtype=np.float32)
    Wph = np.asarray(Wph, dtype=np.float32)
    bh = np.asarray(bh, dtype=np.float32)
    bp = np.asarray(bp, dtype=np.float32)

    guard = _linear_guard(x, Whx, Whh, Wph, bh)
    if guard is not None:
        r = _kernel_linear(x, Whx, Whh, Wph, bh, bp, guard,
                           _want_profile=_want_profile)
        out = r[0] if _want_profile else r
        # post-hoc: approximation bound must be far under the gate
        denom = float(np.abs(out).max())
        if denom > 0 and guard[3] < 2e-3 * denom:
            return r
    return _kernel_tanh(x, Whx, Whh, Wph, bh, bp,
                        _want_profile=_want_profile)
